# revision 2
# baseline (speedup 1.0000x reference)
"""Dilated segment attention on 8 TRN2 NeuronCores (Bass/Tile).

Problem (hardcoded from spec):
  x [2, 8192, 2048] f32, Wqkv [6144, 2048], b_qkv [6144], Wout [2048, 2048],
  b_out [2048].  segment=512, dilation=2 -> 16 segments of L=256 dilated
  tokens per batch; per-segment 16-head attention (hd=128); fused qkv and
  out projections.  Output [2, 4096, 2048] f32.

Sharding: the 32 (batch, segment) instances are independent -> 4 per core.
Host pre-gathers the dilated tokens, pre-transposes/pre-tiles operands and
casts to bf16 (compute precision; measured end-to-end rel err ~5e-3).

Per-core dataflow (all matmuls K=128, bf16):
  qkv proj   : feature-major  qkvT[e, tok] = W-tile.T @ xsT-tile  (accum 16
               d-tiles into 2 psum halves), drained psum->sbuf on the DVE
               (tensor_scalar_add with the per-chunk bias column) so the
               ScalarE queue carries ONLY the softmax exps -- an exp stuck
               behind chunk drains stalls the PE attention stream.
  scores     : scoresT[lk, lq] = kT.T @ qT  (per seg, head; exp is
               layout-agnostic, scores provably in [-6,6] -> no max pass)
  softmax    : exp on ScalarE; row sums via GpSimd partition_all_reduce
               (off the PE -- the old ones-matmul cost ~14us of PE), then
               DVE add halves -> reciprocal -> normalize at the psum->sbuf
               copy of the AV output.
  AV         : outT[hd, lq] = v[lk, hd].T @ expT[lk, lq]
  out proj   : out[l, e] = aT-tile.T @ WoutT-tile  (accum 16 head-tiles,
               token-major, so the HBM store is linear)

Attention for head h-1 is spread across 4 emission slots woven into head
h's q/k projection chunks (pre-q, post-q, and two post-k): each seg's
exp->AV->normalize chain then has a half-chunk (~3.5us) of independent
projection matmuls to hide behind, and the AV psum bank (2 bufs) has two
slots (~7us) to drain before reuse.  scoresT are emitted two slots ahead
of their exp (3 psum bufs) so the ScalarE exp never gates the PE.
b_out is applied on the host (purely linear post-op).
"""

import numpy as np
import ml_dtypes

B = 2
S = 8192
D = 2048
H = 16
HD = 128
SEGMENT = 512
DIL = 2
NSEG = S // SEGMENT          # 16
L = SEGMENT // DIL           # 256 dilated tokens per segment
N_CORES = 8
PAIRS = B * NSEG             # 32 independent (b, n) instances
SPC = PAIRS // N_CORES       # 4 segments per core
TOK = SPC * L                # 1024 tokens per core
DT = D // 128                # 16 contraction tiles
NCHUNK = 3 * D // 128        # 48 qkv feature chunks (16 q, 16 k, 16 v)
SCALE = 1.0 / float(np.sqrt(HD))

_PROGRAM = None


def _build_program():
    import concourse.bass as bass
    import concourse.bacc as bacc
    import concourse.tile as tile
    from concourse import mybir, bass_isa

    BF = mybir.dt.bfloat16
    F32 = mybir.dt.float32
    ts = bass.ts
    ADD = bass_isa.ReduceOp.add

    nc = bacc.Bacc("TRN2", target_bir_lowering=False, debug=False,
                   num_devices=N_CORES)

    xst_d = nc.dram_tensor("xst", [128, DT * TOK], BF, kind="ExternalInput")
    wqkv_d = nc.dram_tensor("wqkv_t", [NCHUNK, 128, DT * 128], BF,
                            kind="ExternalInput")
    wout_d = nc.dram_tensor("wout_t", [4, 128, DT * 512], BF, kind="ExternalInput")
    bq_d = nc.dram_tensor("bq_t", [128, NCHUNK], F32, kind="ExternalInput")
    out_d = nc.dram_tensor("out", [TOK, D], F32, kind="ExternalOutput")

    with tile.TileContext(nc) as tc:
        with (
            tc.tile_pool(name="const", bufs=1) as const_p,
            tc.tile_pool(name="big", bufs=1) as big_p,
            tc.tile_pool(name="wq", bufs=6) as w_p,
            tc.tile_pool(name="qk", bufs=4) as qk_p,
            tc.tile_pool(name="vt", bufs=2) as vt_p,
            tc.tile_pool(name="ex", bufs=4) as ex_p,
            tc.tile_pool(name="nr", bufs=2) as nr_p,
            tc.tile_pool(name="ou", bufs=2) as ou_p,
            tc.tile_pool(name="pp", bufs=3, space="PSUM") as pp_p,
            tc.tile_pool(name="pa", bufs=3, space="PSUM") as pa_p,
        ):
            # First weight chunk DMA leads everything: the first LDWEIGHTS
            # was measured waiting ~5us on it behind other queued DMAs.
            first_w = w_p.tile([128, DT * 128], BF, tag="w", name="first_w")
            nc.sync.dma_start(out=first_w[:], in_=wqkv_d[32])

            # One linear 512KB DMA per weight chunk and 4 x 1MB for xst:
            # the Sync sequencer dispatches each DMA in ~600ns, so hundreds
            # of small tile DMAs would serialize into multi-us delivery
            # latency at every chunk boundary.
            xst_sb = big_p.tile([128, DT, TOK], BF)
            for kk in range(4):
                nc.sync.dma_start(
                    out=xst_sb[:, 4 * kk:4 * (kk + 1), :],
                    in_=xst_d[:, 4 * kk * TOK:4 * (kk + 1) * TOK],
                )
            bq_sb = const_p.tile([128, NCHUNK], F32)
            nc.sync.dma_start(out=bq_sb[:], in_=bq_d[:])

            vtok_sb = big_p.tile([128, H, SPC * 2, 128], BF)
            aT_sb = big_p.tile([128, SPC, H, L], BF)

            def proj_mms(c, wck=None):
                """Emit the 32 accumulation matmuls for qkv chunk c; return
                the two psum halves (drain separately via drain_ops)."""
                if wck is None:
                    wck = w_p.tile([128, DT * 128], BF, tag="w")
                    nc.sync.dma_start(out=wck[:], in_=wqkv_d[c])
                pss = [pp_p.tile([128, 512], F32, tag="pp", bufs=3,
                                 name=f"ps{half}") for half in range(2)]
                for dt in range(DT):
                    for half in range(2):
                        nc.tensor.matmul(
                            pss[half][:],
                            wck[:, ts(dt, 128)],
                            xst_sb[:, dt, ts(half, 512)],
                            start=(dt == 0),
                            stop=(dt == DT - 1),
                        )
                return pss

            def drain(c, pss, out_tile, lo, hi):
                """psum->sbuf bf16 copy of token range [lo,hi) + bias (DVE)."""
                half, hw = lo // 512, 512
                assert hi - lo <= hw and lo // 512 == (hi - 1) // 512
                nc.vector.tensor_scalar_add(
                    out_tile[:, lo:hi],
                    pss[half][:, lo - half * hw:hi - half * hw],
                    bq_sb[:, c:c + 1],
                )

            # ---- v projection (feature-major) + transpose to token-major ----
            # One transposing DMA per head (xbar transpose, ~261GB/s): row
            # tok = tc*128+p of vt.T lands at vtok[p, tc, :], exactly the AV
            # stationary layout.  Emitted one chunk behind the projection so
            # the DMA never waits on the psum->sbuf drain in flight.
            def v_transposes(h, vt_tile):
                nc.sync.dma_start(out=vtok_sb[:, h, :, :], in_=vt_tile[:],
                                  transpose=True)

            prev_v = None
            for h in range(H):
                vt_tile = vt_p.tile([128, TOK], BF, tag="vt")
                pss = proj_mms(32 + h, wck=first_w if h == 0 else None)
                drain(32 + h, pss, vt_tile, 0, 512)
                drain(32 + h, pss, vt_tile, 512, 1024)
                if prev_v is not None:
                    v_transposes(h - 1, prev_v)
                prev_v = vt_tile
            v_transposes(H - 1, prev_v)

            # ---- per-head q/k projection + spread attention ----
            scts = {}

            def sct(hp, seg, qh, kh):
                """scoresT[lk, lq] for (head hp, seg): 2 matmuls, one per
                128-wide lk chunk."""
                t = pa_p.tile([128, 2, L], F32, tag="pa", bufs=3, name="scT")
                for lkc in range(2):
                    nc.tensor.matmul(
                        t[:, lkc, :],
                        kh[:, seg * L + lkc * 128: seg * L + (lkc + 1) * 128],
                        qh[:, seg * L:(seg + 1) * L],
                    )
                scts[(hp, seg)] = t

            def grp(hp, seg):
                """exp -> AV -> (row sums off-PE) -> normalized aT store."""
                scT = scts.pop((hp, seg))
                e_t = ex_p.tile([128, 2, L], BF, tag="ex")
                nc.scalar.activation(
                    out=e_t[:],
                    in_=scT[:],
                    func=mybir.ActivationFunctionType.Exp,
                    scale=SCALE,
                )
                avs = pa_p.tile([128, 2, L], F32, tag="pav", bufs=2,
                                name="avs")
                for lkc in range(2):
                    nc.tensor.matmul(
                        avs[:, 0, :],
                        vtok_sb[:, hp, seg * 2 + lkc, :],
                        e_t[:, lkc, :],
                        start=(lkc == 0),
                        stop=(lkc == 1),
                    )
                er = nr_p.tile([128, 2, L], F32, tag="er")
                nc.gpsimd.partition_all_reduce(er[:], e_t[:], 128, ADD)
                sm = nr_p.tile([128, L], F32, tag="sm")
                nc.vector.tensor_add(sm[:], er[:, 0, :], er[:, 1, :])
                inv = nr_p.tile([128, L], F32, tag="inv")
                nc.vector.reciprocal_approx_fast(out=inv[:], in_=sm[:])
                nc.vector.tensor_mul(aT_sb[:, seg, hp, :], avs[:, 0, :], inv[:])

            qks = {}
            for h in range(H):
                hp = h - 1
                qh = qk_p.tile([128, TOK], BF, tag="qk")
                kh = qk_p.tile([128, TOK], BF, tag="qk")
                qks[h] = (qh, kh)
                pq, pk = qks.get(hp, (None, None))

                # S_A: next-next scores first (keeps PE fed), then seg 0
                if hp >= 0:
                    sct(hp, 2, pq, pk)
                    grp(hp, 0)
                ps_q = proj_mms(h)
                drain(h, ps_q, qh, 0, 512)
                drain(h, ps_q, qh, 512, 1024)
                # S_C
                if hp >= 0:
                    sct(hp, 3, pq, pk)
                    grp(hp, 1)
                ps_k = proj_mms(16 + h)
                # kh drains split so seg-0/1 slices land first: the scores
                # of head h's segs 0/1 (emitted below) read them ~1us later.
                drain(16 + h, ps_k, kh, 0, 256)
                drain(16 + h, ps_k, kh, 256, 512)
                # S_E
                if hp >= 0:
                    grp(hp, 2)
                drain(16 + h, ps_k, kh, 512, 1024)
                # S_F
                if hp >= 0:
                    grp(hp, 3)
                sct(h, 0, qh, kh)
                sct(h, 1, qh, kh)
                if hp >= 0:
                    del qks[hp]

            # tail: head 15's attention (no projection left to weave into)
            pq, pk = qks[H - 1]
            sct(H - 1, 2, pq, pk)
            grp(H - 1, 0)
            sct(H - 1, 3, pq, pk)
            grp(H - 1, 1)
            grp(H - 1, 2)
            grp(H - 1, 3)

            # ---- output projection (token-major) ----
            # Wout is streamed in four 2MB e-quarters (one linear DMA each);
            # Sync runs ahead of the PE so each lands before its matmuls.
            for eq in range(4):
                wq_t = w_p.tile([128, DT, 512], BF, tag="wo", bufs=2,
                                name="wq_t")
                nc.sync.dma_start(out=wq_t[:], in_=wout_d[eq])
                for lc in range(TOK // 128):
                    seg, lqc = lc // 2, lc % 2
                    last = (eq == 3 and lc == TOK // 128 - 1)
                    # the final drain chain defines kernel end: split the
                    # last tile in half so the first store starts earlier.
                    splits = ((0, 256), (256, 512)) if last else ((0, 512),)
                    for lo, hi in splits:
                        po = pp_p.tile([128, hi - lo], F32, tag="pp", bufs=3,
                                       name="po")
                        for dt in range(DT):
                            nc.tensor.matmul(
                                po[:],
                                aT_sb[:, seg, dt, ts(lqc, 128)],
                                wq_t[:, dt, lo:hi],
                                start=(dt == 0),
                                stop=(dt == DT - 1),
                            )
                        ob = ou_p.tile([128, hi - lo], F32, tag="ou")
                        nc.vector.tensor_copy(out=ob[:], in_=po[:])
                        nc.sync.dma_start(
                            out=out_d[lc * 128:(lc + 1) * 128,
                                      eq * 512 + lo:eq * 512 + hi],
                            in_=ob[:],
                        )

    nc.compile()
    _dedupe_ldweights(nc)
    return nc


def _dedupe_ldweights(nc):
    """Drop InstLdweights whose weights are already resident in the PE array.

    tile_legalize emits one LDWEIGHTS per matmul; consecutive matmuls that
    share the stationary operand (projection token-halves, out-proj eq
    pairs) reload identical weights, costing ~97ns of PE pipe each.  Walk
    each block's PE stream tracking the loaded-weights key and delete
    reloads.  Only semaphore-free LDWEIGHTS are dropped, so the sync graph
    is untouched; EVENT_SEMAPHORE/DRAIN between pairs don't disturb the
    array, any other PE instruction conservatively invalidates the key.
    """
    from concourse import mybir

    PE = mybir.EngineType.PE
    dropped = 0
    for f in nc.m.functions:
        for blk in f.blocks:
            insts = blk.instructions
            loaded = None
            to_drop = []
            for idx, x in enumerate(insts):
                if getattr(x, "engine", None) != PE:
                    continue
                nm = type(x).__name__
                if nm == "InstLdweights":
                    si = x.sync_info
                    clean = si is None or (not si.on_wait and not si.on_update)
                    key = (str(x.ins[0]), str(x.is_transpose),
                           str(x.perf_mode), str(x.tile_position))
                    if clean and loaded == key:
                        to_drop.append(idx)
                    else:
                        loaded = key
                elif nm == "InstMatmult":
                    continue
                elif nm in ("InstEventSemaphore", "InstDrain"):
                    continue
                else:
                    loaded = None
            for idx in reversed(to_drop):
                del insts[idx]
            blk.instructions = insts
            dropped += len(to_drop)
    return dropped


def get_program():
    global _PROGRAM
    if _PROGRAM is None:
        _PROGRAM = _build_program()
    return _PROGRAM


def make_in_maps(x, Wqkv, b_qkv):
    """Host-side shard + layout prep (bf16 casts, transposes, tiling)."""
    bf16 = ml_dtypes.bfloat16
    x = np.asarray(x, dtype=np.float32)
    Wqkv = np.asarray(Wqkv, dtype=np.float32)
    b_qkv = np.asarray(b_qkv, dtype=np.float32)

    xs = x.reshape(B, NSEG, SEGMENT, D)[:, :, ::DIL, :]     # [2,16,256,2048]
    xs_flat = xs.reshape(PAIRS, L, D)

    # lhsT tiles packed partition-major: wt[c, p, dt*128+j] = WqkvT[dt*128+p,
    # c*128+j] so one chunk is a single linear per-partition DMA.
    wt = np.ascontiguousarray(
        Wqkv.reshape(NCHUNK, 128, DT, 128).transpose(0, 3, 2, 1)
        .reshape(NCHUNK, 128, DT * 128)
    ).astype(bf16)                                          # [48,128,2048]
    bqt = np.ascontiguousarray(b_qkv.reshape(NCHUNK, 128).T)  # [128,48] f32

    in_maps = []
    for i in range(N_CORES):
        tok = xs_flat[SPC * i:SPC * (i + 1)].reshape(TOK, D)
        xst = np.ascontiguousarray(
            tok.T.reshape(DT, 128, TOK).transpose(1, 0, 2)
            .reshape(128, DT * TOK)).astype(bf16)
        in_maps.append({"xst": xst, "wqkv_t": wt, "bq_t": bqt})
    return in_maps


def make_wout_tiled(Wout):
    Wout = np.asarray(Wout, dtype=np.float32)
    # [eq, p, dt*512+j] = Wout[eq*512+j, dt*128+p]: one linear DMA/quarter
    return np.ascontiguousarray(
        Wout.T.reshape(DT, 128, 4, 512).transpose(2, 1, 0, 3)
        .reshape(4, 128, DT * 512)).astype(ml_dtypes.bfloat16)


def kernel(x, Wqkv, b_qkv, Wout, b_out):
    from concourse import bass_utils

    nc = get_program()
    in_maps = make_in_maps(x, Wqkv, b_qkv)
    wot = make_wout_tiled(Wout)
    for m in in_maps:
        m["wout_t"] = wot

    res = bass_utils.run_bass_kernel_spmd(
        nc, in_maps, core_ids=list(range(N_CORES)))
    outs = [res.results[i]["out"] for i in range(N_CORES)]
    full = np.concatenate(outs, axis=0) + np.asarray(b_out, dtype=np.float32)
    return np.ascontiguousarray(full.reshape(B, NSEG * L, D), dtype=np.float32)


# revision 6
# speedup vs baseline: 1.4690x; 1.4690x over previous
"""Dilated segment attention on 8 TRN2 NeuronCores (Bass/Tile).

Problem (hardcoded from spec):
  x [2, 8192, 2048] f32, Wqkv [6144, 2048], b_qkv [6144], Wout [2048, 2048],
  b_out [2048].  segment=512, dilation=2 -> 16 segments of L=256 dilated
  tokens per batch; per-segment 16-head attention (hd=128); fused qkv and
  out projections.  Output [2, 4096, 2048] f32.

Sharding: the 32 (batch, segment) instances are independent -> 4 per core.
Host pre-gathers the dilated tokens, pre-transposes/pre-tiles operands and
casts to bf16 (compute precision; measured end-to-end rel err ~5e-3).

Per-core dataflow (all matmuls K=128, bf16):
  qkv proj   : feature-major  qkvT[e, tok] = W-tile.T @ xsT-tile  (accum 16
               d-tiles into 2 psum halves), drained psum->sbuf on the DVE
               (tensor_scalar_add with the per-chunk bias column) so the
               ScalarE queue carries ONLY the softmax exps -- an exp stuck
               behind chunk drains stalls the PE attention stream.
  scores     : scoresT[lk, lq] = kT.T @ qT  (per seg, head; exp is
               layout-agnostic, scores provably in [-6,6] -> no max pass)
  softmax    : exp on ScalarE; row sums via a ones-matmul on the PE
               (GpSimd partition_all_reduce was tried and measured 4.3us
               per [128,512] op -- saturates GpSimd and stalls the PE via
               the AV psum-bank chain), then DVE reciprocal -> GpSimd
               partition broadcast -> normalize at the psum->sbuf copy of
               the AV output.
  AV         : outT[hd, lq] = v[lk, hd].T @ expT[lk, lq]
  out proj   : out[l, e] = aT-tile.T @ WoutT-tile  (accum 16 head-tiles,
               token-major, so the HBM store is linear)

Attention for head h-1 is spread across 4 emission slots woven into head
h's q/k projection chunks (pre-q, post-q, and two post-k): each seg's
exp->AV->normalize chain then has a half-chunk (~3.5us) of independent
projection matmuls to hide behind, and the AV psum bank (2 bufs) has two
slots (~7us) to drain before reuse.  scoresT are emitted two slots ahead
of their exp (3 psum bufs) so the ScalarE exp never gates the PE.
b_out is applied on the host (purely linear post-op).
"""

import numpy as np
import ml_dtypes

B = 2
S = 8192
D = 2048
H = 16
HD = 128
SEGMENT = 512
DIL = 2
NSEG = S // SEGMENT          # 16
L = SEGMENT // DIL           # 256 dilated tokens per segment
N_CORES = 8
PAIRS = B * NSEG             # 32 independent (b, n) instances
SPC = PAIRS // N_CORES       # 4 segments per core
TOK = SPC * L                # 1024 tokens per core
DT = D // 128                # 16 contraction tiles
NCHUNK = 3 * D // 128        # 48 qkv feature chunks (16 q, 16 k, 16 v)
SCALE = 1.0 / float(np.sqrt(HD))

_PROGRAM = None


def _build_program():
    import concourse.bass as bass
    import concourse.bacc as bacc
    import concourse.tile as tile
    from concourse import mybir

    BF = mybir.dt.bfloat16
    F32 = mybir.dt.float32
    ts = bass.ts

    nc = bacc.Bacc("TRN2", target_bir_lowering=False, debug=False,
                   num_devices=N_CORES)

    xst_d = nc.dram_tensor("xst", [128, DT * TOK], BF, kind="ExternalInput")
    wqkv_d = nc.dram_tensor("wqkv_t", [NCHUNK, 128, DT * 128], BF,
                            kind="ExternalInput")
    wout_d = nc.dram_tensor("wout_t", [4, 128, DT * 512], BF, kind="ExternalInput")
    bq_d = nc.dram_tensor("bq_t", [128, NCHUNK], F32, kind="ExternalInput")
    out_d = nc.dram_tensor("out", [TOK, D], F32, kind="ExternalOutput")

    with tile.TileContext(nc) as tc:
        with (
            tc.tile_pool(name="const", bufs=1) as const_p,
            tc.tile_pool(name="big", bufs=1) as big_p,
            tc.tile_pool(name="wq", bufs=6) as w_p,
            tc.tile_pool(name="qk", bufs=4) as qk_p,
            tc.tile_pool(name="vt", bufs=2) as vt_p,
            tc.tile_pool(name="ex", bufs=4) as ex_p,
            tc.tile_pool(name="nr", bufs=2) as nr_p,
            tc.tile_pool(name="ou", bufs=2) as ou_p,
            tc.tile_pool(name="pp", bufs=3, space="PSUM") as pp_p,
            tc.tile_pool(name="pa", bufs=3, space="PSUM") as pa_p,
        ):
            # First weight chunk DMA leads everything: the first LDWEIGHTS
            # was measured waiting ~5us on it behind other queued DMAs.
            first_w = w_p.tile([128, DT * 128], BF, tag="w", name="first_w")
            nc.sync.dma_start(out=first_w[:], in_=wqkv_d[32])

            # One linear 512KB DMA per weight chunk and 4 x 1MB for xst:
            # the Sync sequencer dispatches each DMA in ~600ns, so hundreds
            # of small tile DMAs would serialize into multi-us delivery
            # latency at every chunk boundary.
            xst_sb = big_p.tile([128, DT, TOK], BF)
            for kk in range(4):
                nc.sync.dma_start(
                    out=xst_sb[:, 4 * kk:4 * (kk + 1), :],
                    in_=xst_d[:, 4 * kk * TOK:4 * (kk + 1) * TOK],
                )
            bq_sb = const_p.tile([128, NCHUNK], F32)
            nc.sync.dma_start(out=bq_sb[:], in_=bq_d[:])
            ones = const_p.tile([128, 1], BF)
            nc.gpsimd.memset(ones[:], 1.0)

            vtok_sb = big_p.tile([128, H, SPC * 2, 128], BF)
            aT_sb = big_p.tile([128, SPC, H, L], BF)

            def proj_mms(c, wck=None):
                """Emit the 32 accumulation matmuls for qkv chunk c; return
                the two psum halves (drain separately via drain_ops)."""
                if wck is None:
                    wck = w_p.tile([128, DT * 128], BF, tag="w")
                    nc.sync.dma_start(out=wck[:], in_=wqkv_d[c])
                pss = [pp_p.tile([128, 512], F32, tag="pp", bufs=3,
                                 name=f"ps{half}") for half in range(2)]
                for dt in range(DT):
                    for half in range(2):
                        nc.tensor.matmul(
                            pss[half][:],
                            wck[:, ts(dt, 128)],
                            xst_sb[:, dt, ts(half, 512)],
                            start=(dt == 0),
                            stop=(dt == DT - 1),
                        )
                return pss

            def drain(c, pss, out_tile, lo, hi):
                """psum->sbuf bf16 copy of token range [lo,hi) + bias (DVE)."""
                half, hw = lo // 512, 512
                assert hi - lo <= hw and lo // 512 == (hi - 1) // 512
                nc.vector.tensor_scalar_add(
                    out_tile[:, lo:hi],
                    pss[half][:, lo - half * hw:hi - half * hw],
                    bq_sb[:, c:c + 1],
                )

            # ---- v projection (feature-major) + transpose to token-major ----
            # One transposing DMA per head (xbar transpose, ~261GB/s): row
            # tok = tc*128+p of vt.T lands at vtok[p, tc, :], exactly the AV
            # stationary layout.  Emitted one chunk behind the projection so
            # the DMA never waits on the psum->sbuf drain in flight.
            def v_transposes(h, vt_tile):
                nc.sync.dma_start(out=vtok_sb[:, h, :, :], in_=vt_tile[:],
                                  transpose=True)

            prev_v = None
            for h in range(H):
                vt_tile = vt_p.tile([128, TOK], BF, tag="vt")
                pss = proj_mms(32 + h, wck=first_w if h == 0 else None)
                drain(32 + h, pss, vt_tile, 0, 512)
                drain(32 + h, pss, vt_tile, 512, 1024)
                if prev_v is not None:
                    v_transposes(h - 1, prev_v)
                prev_v = vt_tile
            v_transposes(H - 1, prev_v)

            # ---- per-head q/k projection + spread attention ----
            scts = {}

            def sct(hp, seg, qh, kh):
                """scoresT[lk, lq] for (head hp, seg): 2 matmuls, one per
                128-wide lk chunk."""
                t = pa_p.tile([128, 2, L], F32, tag="pa", bufs=3, name="scT")
                for lkc in range(2):
                    nc.tensor.matmul(
                        t[:, lkc, :],
                        kh[:, seg * L + lkc * 128: seg * L + (lkc + 1) * 128],
                        qh[:, seg * L:(seg + 1) * L],
                    )
                scts[(hp, seg)] = t

            def grp(hp, seg):
                """exp -> row sums + AV -> normalized aT store.
                av ([:, 0, :]) and the softmax sums row ([0:1, 1, :]) share
                one PSUM bank; Tile serializes the cross-use."""
                scT = scts.pop((hp, seg))
                e_t = ex_p.tile([128, 2, L], BF, tag="ex")
                nc.scalar.activation(
                    out=e_t[:],
                    in_=scT[:],
                    func=mybir.ActivationFunctionType.Exp,
                    scale=SCALE,
                )
                avs = pa_p.tile([128, 2, L], F32, tag="pav", bufs=2,
                                name="avs")
                for lkc in range(2):
                    nc.tensor.matmul(
                        avs[0:1, 1, :],
                        ones[:],
                        e_t[:, lkc, :],
                        start=(lkc == 0),
                        stop=(lkc == 1),
                    )
                for lkc in range(2):
                    nc.tensor.matmul(
                        avs[:, 0, :],
                        vtok_sb[:, hp, seg * 2 + lkc, :],
                        e_t[:, lkc, :],
                        start=(lkc == 0),
                        stop=(lkc == 1),
                    )
                inv = nr_p.tile([1, L], F32, tag="st")
                nc.vector.reciprocal_approx_fast(out=inv[:], in_=avs[0:1, 1, :])
                invB = nr_p.tile([128, L], F32, tag="invb")
                nc.gpsimd.partition_broadcast(invB[:], inv[:])
                nc.vector.tensor_mul(aT_sb[:, seg, hp, :], avs[:, 0, :], invB[:])

            qks = {}
            for h in range(H):
                hp = h - 1
                qh = qk_p.tile([128, TOK], BF, tag="qk")
                kh = qk_p.tile([128, TOK], BF, tag="qk")
                qks[h] = (qh, kh)
                pq, pk = qks.get(hp, (None, None))

                # S_A: next-next scores first (keeps PE fed), then seg 0
                if hp >= 0:
                    sct(hp, 2, pq, pk)
                    grp(hp, 0)
                ps_q = proj_mms(h)
                drain(h, ps_q, qh, 0, 512)
                drain(h, ps_q, qh, 512, 1024)
                # S_C
                if hp >= 0:
                    sct(hp, 3, pq, pk)
                    grp(hp, 1)
                ps_k = proj_mms(16 + h)
                # kh drains split so seg-0/1 slices land first: the scores
                # of head h's segs 0/1 (emitted below) read them ~1us later.
                drain(16 + h, ps_k, kh, 0, 256)
                drain(16 + h, ps_k, kh, 256, 512)
                # S_E
                if hp >= 0:
                    grp(hp, 2)
                drain(16 + h, ps_k, kh, 512, 1024)
                # S_F
                if hp >= 0:
                    grp(hp, 3)
                sct(h, 0, qh, kh)
                sct(h, 1, qh, kh)
                if hp >= 0:
                    del qks[hp]

            # tail: head 15's attention (no projection left to weave into)
            pq, pk = qks[H - 1]
            sct(H - 1, 2, pq, pk)
            grp(H - 1, 0)
            sct(H - 1, 3, pq, pk)
            grp(H - 1, 1)
            grp(H - 1, 2)
            grp(H - 1, 3)

            # ---- output projection (token-major) ----
            # Wout is streamed in four 2MB e-quarters (one linear DMA each);
            # Sync runs ahead of the PE so each lands before its matmuls.
            for eq in range(4):
                wq_t = w_p.tile([128, DT, 512], BF, tag="wo", bufs=2,
                                name="wq_t")
                nc.sync.dma_start(out=wq_t[:], in_=wout_d[eq])
                for lc in range(TOK // 128):
                    seg, lqc = lc // 2, lc % 2
                    last = (eq == 3 and lc == TOK // 128 - 1)
                    # the final drain chain defines kernel end: split the
                    # last tile in half so the first store starts earlier.
                    splits = ((0, 256), (256, 512)) if last else ((0, 512),)
                    for lo, hi in splits:
                        po = pp_p.tile([128, hi - lo], F32, tag="pp", bufs=3,
                                       name="po")
                        for dt in range(DT):
                            nc.tensor.matmul(
                                po[:],
                                aT_sb[:, seg, dt, ts(lqc, 128)],
                                wq_t[:, dt, lo:hi],
                                start=(dt == 0),
                                stop=(dt == DT - 1),
                            )
                        ob = ou_p.tile([128, hi - lo], F32, tag="ou")
                        nc.vector.tensor_copy(out=ob[:], in_=po[:])
                        nc.sync.dma_start(
                            out=out_d[lc * 128:(lc + 1) * 128,
                                      eq * 512 + lo:eq * 512 + hi],
                            in_=ob[:],
                        )

    nc.compile()
    _dedupe_ldweights(nc)
    return nc


def _dedupe_ldweights(nc):
    """Drop InstLdweights whose weights are already resident in the PE array.

    tile_legalize emits one LDWEIGHTS per matmul; consecutive matmuls that
    share the stationary operand (projection token-halves, out-proj eq
    pairs) reload identical weights, costing ~97ns of PE pipe each.  Walk
    each block's PE stream tracking the loaded-weights key and delete
    reloads.  Only semaphore-free LDWEIGHTS are dropped, so the sync graph
    is untouched; EVENT_SEMAPHORE/DRAIN between pairs don't disturb the
    array, any other PE instruction conservatively invalidates the key.
    """
    from concourse import mybir

    PE = mybir.EngineType.PE
    dropped = 0
    for f in nc.m.functions:
        for blk in f.blocks:
            insts = blk.instructions
            loaded = None
            to_drop = []
            for idx, x in enumerate(insts):
                if getattr(x, "engine", None) != PE:
                    continue
                nm = type(x).__name__
                if nm == "InstLdweights":
                    si = x.sync_info
                    clean = si is None or (not si.on_wait and not si.on_update)
                    key = (str(x.ins[0]), str(x.is_transpose),
                           str(x.perf_mode), str(x.tile_position))
                    if clean and loaded == key:
                        to_drop.append(idx)
                    else:
                        loaded = key
                elif nm == "InstMatmult":
                    continue
                elif nm in ("InstEventSemaphore", "InstDrain"):
                    continue
                else:
                    loaded = None
            for idx in reversed(to_drop):
                del insts[idx]
            blk.instructions = insts
            dropped += len(to_drop)
    return dropped


def get_program():
    global _PROGRAM
    if _PROGRAM is None:
        _PROGRAM = _build_program()
    return _PROGRAM


def make_in_maps(x, Wqkv, b_qkv):
    """Host-side shard + layout prep (bf16 casts, transposes, tiling)."""
    bf16 = ml_dtypes.bfloat16
    x = np.asarray(x, dtype=np.float32)
    Wqkv = np.asarray(Wqkv, dtype=np.float32)
    b_qkv = np.asarray(b_qkv, dtype=np.float32)

    xs = x.reshape(B, NSEG, SEGMENT, D)[:, :, ::DIL, :]     # [2,16,256,2048]
    xs_flat = xs.reshape(PAIRS, L, D)

    # lhsT tiles packed partition-major: wt[c, p, dt*128+j] = WqkvT[dt*128+p,
    # c*128+j] so one chunk is a single linear per-partition DMA.
    wt = np.ascontiguousarray(
        Wqkv.reshape(NCHUNK, 128, DT, 128).transpose(0, 3, 2, 1)
        .reshape(NCHUNK, 128, DT * 128)
    ).astype(bf16)                                          # [48,128,2048]
    bqt = np.ascontiguousarray(b_qkv.reshape(NCHUNK, 128).T)  # [128,48] f32

    in_maps = []
    for i in range(N_CORES):
        tok = xs_flat[SPC * i:SPC * (i + 1)].reshape(TOK, D)
        xst = np.ascontiguousarray(
            tok.T.reshape(DT, 128, TOK).transpose(1, 0, 2)
            .reshape(128, DT * TOK)).astype(bf16)
        in_maps.append({"xst": xst, "wqkv_t": wt, "bq_t": bqt})
    return in_maps


def make_wout_tiled(Wout):
    Wout = np.asarray(Wout, dtype=np.float32)
    # [eq, p, dt*512+j] = Wout[eq*512+j, dt*128+p]: one linear DMA/quarter
    return np.ascontiguousarray(
        Wout.T.reshape(DT, 128, 4, 512).transpose(2, 1, 0, 3)
        .reshape(4, 128, DT * 512)).astype(ml_dtypes.bfloat16)


def kernel(x, Wqkv, b_qkv, Wout, b_out):
    from concourse import bass_utils

    nc = get_program()
    in_maps = make_in_maps(x, Wqkv, b_qkv)
    wot = make_wout_tiled(Wout)
    for m in in_maps:
        m["wout_t"] = wot

    res = bass_utils.run_bass_kernel_spmd(
        nc, in_maps, core_ids=list(range(N_CORES)))
    outs = [res.results[i]["out"] for i in range(N_CORES)]
    full = np.concatenate(outs, axis=0) + np.asarray(b_out, dtype=np.float32)
    return np.ascontiguousarray(full.reshape(B, NSEG * L, D), dtype=np.float32)


# revision 9
# speedup vs baseline: 1.4943x; 1.0172x over previous
"""Dilated segment attention on 8 TRN2 NeuronCores (Bass/Tile).

Problem (hardcoded from spec):
  x [2, 8192, 2048] f32, Wqkv [6144, 2048], b_qkv [6144], Wout [2048, 2048],
  b_out [2048].  segment=512, dilation=2 -> 16 segments of L=256 dilated
  tokens per batch; per-segment 16-head attention (hd=128); fused qkv and
  out projections.  Output [2, 4096, 2048] f32.

Sharding: the 32 (batch, segment) instances are independent -> 4 per core.
Host pre-gathers the dilated tokens, pre-transposes/pre-tiles operands and
casts to bf16 (compute precision; measured end-to-end rel err ~5e-3).

Per-core dataflow (all matmuls K=128, bf16):
  qkv proj   : feature-major  qkvT[e, tok] = W-tile.T @ xsT-tile  (accum 16
               d-tiles into 2 psum halves), drained psum->sbuf on the DVE
               (tensor_scalar_add with the per-chunk bias column) so the
               ScalarE queue carries ONLY the softmax exps -- an exp stuck
               behind chunk drains stalls the PE attention stream.
  scores     : scoresT[lk, lq] = kT.T @ qT  (per seg, head; exp is
               layout-agnostic, scores provably in [-6,6] -> no max pass)
  softmax    : exp on ScalarE; row sums via a ones-matmul on the PE
               (GpSimd partition_all_reduce was tried and measured 4.3us
               per [128,512] op -- saturates GpSimd and stalls the PE via
               the AV psum-bank chain), then DVE reciprocal -> GpSimd
               partition broadcast -> normalize at the psum->sbuf copy of
               the AV output.
  AV         : outT[hd, lq] = v[lk, hd].T @ expT[lk, lq]
  out proj   : out[l, e] = aT-tile.T @ WoutT-tile  (accum 16 head-tiles,
               token-major, so the HBM store is linear)

Attention for head h-1 is spread across 4 emission slots woven into head
h's q/k projection chunks (pre-q, post-q, and two post-k): each seg's
exp->AV->normalize chain then has a half-chunk (~3.5us) of independent
projection matmuls to hide behind, and the AV psum bank (2 bufs) has two
slots (~7us) to drain before reuse.  scoresT are emitted two slots ahead
of their exp (3 psum bufs) so the ScalarE exp never gates the PE.
b_out is applied on the host (purely linear post-op).
"""

import numpy as np
import ml_dtypes

B = 2
S = 8192
D = 2048
H = 16
HD = 128
SEGMENT = 512
DIL = 2
NSEG = S // SEGMENT          # 16
L = SEGMENT // DIL           # 256 dilated tokens per segment
N_CORES = 8
PAIRS = B * NSEG             # 32 independent (b, n) instances
SPC = PAIRS // N_CORES       # 4 segments per core
TOK = SPC * L                # 1024 tokens per core
DT = D // 128                # 16 contraction tiles
NCHUNK = 3 * D // 128        # 48 qkv feature chunks (16 q, 16 k, 16 v)
SCALE = 1.0 / float(np.sqrt(HD))

_PROGRAM = None


def _build_program():
    import concourse.bass as bass
    import concourse.bacc as bacc
    import concourse.tile as tile
    from concourse import mybir

    BF = mybir.dt.bfloat16
    F32 = mybir.dt.float32
    ts = bass.ts

    nc = bacc.Bacc("TRN2", target_bir_lowering=False, debug=False,
                   num_devices=N_CORES)

    xst_d = nc.dram_tensor("xst", [128, DT * TOK], BF, kind="ExternalInput")
    wqkv_d = nc.dram_tensor("wqkv_t", [NCHUNK, 128, DT * 128], BF,
                            kind="ExternalInput")
    wout_d = nc.dram_tensor("wout_t", [4, 128, DT * 512], BF, kind="ExternalInput")
    bq_d = nc.dram_tensor("bq_t", [128, NCHUNK], F32, kind="ExternalInput")
    out_d = nc.dram_tensor("out", [TOK, D], F32, kind="ExternalOutput")

    with tile.TileContext(nc) as tc:
        with (
            tc.tile_pool(name="const", bufs=1) as const_p,
            tc.tile_pool(name="big", bufs=1) as big_p,
            tc.tile_pool(name="wq", bufs=6) as w_p,
            tc.tile_pool(name="qk", bufs=4) as qk_p,
            tc.tile_pool(name="vt", bufs=2) as vt_p,
            tc.tile_pool(name="ex", bufs=4) as ex_p,
            tc.tile_pool(name="nr", bufs=2) as nr_p,
            tc.tile_pool(name="ou", bufs=2) as ou_p,
            tc.tile_pool(name="pp", bufs=3, space="PSUM") as pp_p,
            tc.tile_pool(name="pa", bufs=3, space="PSUM") as pa_p,
        ):
            # Two HWDGE queues: weight chunks stream on Sync; xst, the vtok
            # transposes and the output stores ride the Activation queue
            # (idle for DMA otherwise).  Keeping the 1.3us transposes off
            # the Sync queue stops them from delaying weight chunks (and
            # vice versa -- a transpose stuck behind weight DMAs stalled
            # the vt drain -> psum-bank chain ~5us in the single-queue
            # version), and the first weight chunk + first xst quarter now
            # transfer in parallel at startup.
            first_w = w_p.tile([128, DT * 128], BF, tag="w", name="first_w")
            nc.sync.dma_start(out=first_w[:], in_=wqkv_d[32])

            # One linear 512KB DMA per weight chunk and 4 x 1MB for xst:
            # the sequencer dispatches each DMA in ~600ns, so hundreds of
            # small tile DMAs would serialize into multi-us delivery
            # latency at every chunk boundary.
            xst_sb = big_p.tile([128, DT, TOK], BF)
            for kk in range(4):
                nc.scalar.dma_start(
                    out=xst_sb[:, 4 * kk:4 * (kk + 1), :],
                    in_=xst_d[:, 4 * kk * TOK:4 * (kk + 1) * TOK],
                )
            bq_sb = const_p.tile([128, NCHUNK], F32)
            nc.scalar.dma_start(out=bq_sb[:], in_=bq_d[:])
            ones = const_p.tile([128, 1], BF)
            nc.gpsimd.memset(ones[:], 1.0)

            vtok_sb = big_p.tile([128, H, SPC * 2, 128], BF)
            aT_sb = big_p.tile([128, SPC, H, L], BF)

            def proj_mms(c, wck=None):
                """Emit the 32 accumulation matmuls for qkv chunk c; return
                the two psum halves (drain separately via drain_ops)."""
                if wck is None:
                    wck = w_p.tile([128, DT * 128], BF, tag="w")
                    nc.sync.dma_start(out=wck[:], in_=wqkv_d[c])
                pss = [pp_p.tile([128, 512], F32, tag="pp", bufs=3,
                                 name=f"ps{half}") for half in range(2)]
                for dt in range(DT):
                    for half in range(2):
                        nc.tensor.matmul(
                            pss[half][:],
                            wck[:, ts(dt, 128)],
                            xst_sb[:, dt, ts(half, 512)],
                            start=(dt == 0),
                            stop=(dt == DT - 1),
                        )
                return pss

            def drain(c, pss, out_tile, lo, hi):
                """psum->sbuf bf16 copy of token range [lo,hi) + bias (DVE)."""
                half, hw = lo // 512, 512
                assert hi - lo <= hw and lo // 512 == (hi - 1) // 512
                nc.vector.tensor_scalar_add(
                    out_tile[:, lo:hi],
                    pss[half][:, lo - half * hw:hi - half * hw],
                    bq_sb[:, c:c + 1],
                )

            # ---- v projection (feature-major) + transpose to token-major ----
            # One transposing DMA per head (xbar transpose, ~261GB/s): row
            # tok = tc*128+p of vt.T lands at vtok[p, tc, :], exactly the AV
            # stationary layout.  Emitted one chunk behind the projection so
            # the DMA never waits on the psum->sbuf drain in flight.
            def v_transposes(h, vt_tile):
                nc.scalar.dma_start(out=vtok_sb[:, h, :, :], in_=vt_tile[:],
                                    transpose=True)

            prev_v = None
            for h in range(H):
                vt_tile = vt_p.tile([128, TOK], BF, tag="vt", bufs=4)
                pss = proj_mms(32 + h, wck=first_w if h == 0 else None)
                drain(32 + h, pss, vt_tile, 0, 512)
                drain(32 + h, pss, vt_tile, 512, 1024)
                if prev_v is not None:
                    v_transposes(h - 1, prev_v)
                prev_v = vt_tile
            v_transposes(H - 1, prev_v)

            # ---- per-head q/k projection + spread attention ----
            scts = {}

            def sct(hp, seg, qh, kh):
                """scoresT[lk, lq] for (head hp, seg): 2 matmuls, one per
                128-wide lk chunk."""
                t = pa_p.tile([128, 2, L], F32, tag="pa", bufs=3, name="scT")
                for lkc in range(2):
                    nc.tensor.matmul(
                        t[:, lkc, :],
                        kh[:, seg * L + lkc * 128: seg * L + (lkc + 1) * 128],
                        qh[:, seg * L:(seg + 1) * L],
                    )
                scts[(hp, seg)] = t

            def grp(hp, seg):
                """exp -> row sums + AV -> normalized aT store.
                av ([:, 0, :]) and the softmax sums row ([0:1, 1, :]) share
                one PSUM bank; Tile serializes the cross-use."""
                scT = scts.pop((hp, seg))
                e_t = ex_p.tile([128, 2, L], BF, tag="ex")
                nc.scalar.activation(
                    out=e_t[:],
                    in_=scT[:],
                    func=mybir.ActivationFunctionType.Exp,
                    scale=SCALE,
                )
                avs = pa_p.tile([128, 2, L], F32, tag="pav", bufs=2,
                                name="avs")
                for lkc in range(2):
                    nc.tensor.matmul(
                        avs[0:1, 1, :],
                        ones[:],
                        e_t[:, lkc, :],
                        start=(lkc == 0),
                        stop=(lkc == 1),
                    )
                for lkc in range(2):
                    nc.tensor.matmul(
                        avs[:, 0, :],
                        vtok_sb[:, hp, seg * 2 + lkc, :],
                        e_t[:, lkc, :],
                        start=(lkc == 0),
                        stop=(lkc == 1),
                    )
                inv = nr_p.tile([1, L], F32, tag="st")
                nc.vector.reciprocal_approx_fast(out=inv[:], in_=avs[0:1, 1, :])
                invB = nr_p.tile([128, L], F32, tag="invb")
                nc.gpsimd.partition_broadcast(invB[:], inv[:])
                nc.vector.tensor_mul(aT_sb[:, seg, hp, :], avs[:, 0, :], invB[:])

            qks = {}
            for h in range(H):
                hp = h - 1
                qh = qk_p.tile([128, TOK], BF, tag="qk")
                kh = qk_p.tile([128, TOK], BF, tag="qk")
                qks[h] = (qh, kh)
                pq, pk = qks.get(hp, (None, None))

                # S_A: next-next scores first (keeps PE fed), then seg 0
                if hp >= 0:
                    sct(hp, 2, pq, pk)
                    grp(hp, 0)
                ps_q = proj_mms(h)
                drain(h, ps_q, qh, 0, 512)
                drain(h, ps_q, qh, 512, 1024)
                # S_C
                if hp >= 0:
                    sct(hp, 3, pq, pk)
                    grp(hp, 1)
                ps_k = proj_mms(16 + h)
                # kh drains split so seg-0/1 slices land first: the scores
                # of head h's segs 0/1 (emitted below) read them ~1us later.
                drain(16 + h, ps_k, kh, 0, 256)
                drain(16 + h, ps_k, kh, 256, 512)
                # S_E
                if hp >= 0:
                    grp(hp, 2)
                drain(16 + h, ps_k, kh, 512, 1024)
                # S_F
                if hp >= 0:
                    grp(hp, 3)
                sct(h, 0, qh, kh)
                sct(h, 1, qh, kh)
                if hp >= 0:
                    del qks[hp]

            # tail: head 15's attention (no projection left to weave into)
            pq, pk = qks[H - 1]
            sct(H - 1, 2, pq, pk)
            grp(H - 1, 0)
            sct(H - 1, 3, pq, pk)
            grp(H - 1, 1)
            grp(H - 1, 2)
            grp(H - 1, 3)

            # ---- output projection (token-major) ----
            # Wout is streamed in four 2MB e-quarters (one linear DMA each);
            # Sync runs ahead of the PE so each lands before its matmuls.
            for eq in range(4):
                wq_t = w_p.tile([128, DT, 512], BF, tag="wo", bufs=2,
                                name="wq_t")
                nc.sync.dma_start(out=wq_t[:], in_=wout_d[eq])
                for lc in range(TOK // 128):
                    seg, lqc = lc // 2, lc % 2
                    last = (eq == 3 and lc == TOK // 128 - 1)
                    # the final drain chain defines kernel end: split the
                    # last tile in half so the first store starts earlier.
                    splits = ((0, 256), (256, 512)) if last else ((0, 512),)
                    for lo, hi in splits:
                        po = pp_p.tile([128, hi - lo], F32, tag="pp", bufs=3,
                                       name="po")
                        for dt in range(DT):
                            nc.tensor.matmul(
                                po[:],
                                aT_sb[:, seg, dt, ts(lqc, 128)],
                                wq_t[:, dt, lo:hi],
                                start=(dt == 0),
                                stop=(dt == DT - 1),
                            )
                        ob = ou_p.tile([128, hi - lo], F32, tag="ou")
                        nc.vector.tensor_copy(out=ob[:], in_=po[:])
                        nc.scalar.dma_start(
                            out=out_d[lc * 128:(lc + 1) * 128,
                                      eq * 512 + lo:eq * 512 + hi],
                            in_=ob[:],
                        )

    nc.compile()
    _dedupe_ldweights(nc)
    return nc


def _dedupe_ldweights(nc):
    """Drop InstLdweights whose weights are already resident in the PE array.

    tile_legalize emits one LDWEIGHTS per matmul; consecutive matmuls that
    share the stationary operand (projection token-halves, out-proj eq
    pairs) reload identical weights, costing ~97ns of PE pipe each.  Walk
    each block's PE stream tracking the loaded-weights key and delete
    reloads.  Only semaphore-free LDWEIGHTS are dropped, so the sync graph
    is untouched; EVENT_SEMAPHORE/DRAIN between pairs don't disturb the
    array, any other PE instruction conservatively invalidates the key.
    """
    from concourse import mybir

    PE = mybir.EngineType.PE
    dropped = 0
    for f in nc.m.functions:
        for blk in f.blocks:
            insts = blk.instructions
            loaded = None
            to_drop = []
            for idx, x in enumerate(insts):
                if getattr(x, "engine", None) != PE:
                    continue
                nm = type(x).__name__
                if nm == "InstLdweights":
                    si = x.sync_info
                    clean = si is None or (not si.on_wait and not si.on_update)
                    key = (str(x.ins[0]), str(x.is_transpose),
                           str(x.perf_mode), str(x.tile_position))
                    if clean and loaded == key:
                        to_drop.append(idx)
                    else:
                        loaded = key
                elif nm == "InstMatmult":
                    continue
                elif nm in ("InstEventSemaphore", "InstDrain"):
                    continue
                else:
                    loaded = None
            for idx in reversed(to_drop):
                del insts[idx]
            blk.instructions = insts
            dropped += len(to_drop)
    return dropped


def get_program():
    global _PROGRAM
    if _PROGRAM is None:
        _PROGRAM = _build_program()
    return _PROGRAM


def make_in_maps(x, Wqkv, b_qkv):
    """Host-side shard + layout prep (bf16 casts, transposes, tiling)."""
    bf16 = ml_dtypes.bfloat16
    x = np.asarray(x, dtype=np.float32)
    Wqkv = np.asarray(Wqkv, dtype=np.float32)
    b_qkv = np.asarray(b_qkv, dtype=np.float32)

    xs = x.reshape(B, NSEG, SEGMENT, D)[:, :, ::DIL, :]     # [2,16,256,2048]
    xs_flat = xs.reshape(PAIRS, L, D)

    # lhsT tiles packed partition-major: wt[c, p, dt*128+j] = WqkvT[dt*128+p,
    # c*128+j] so one chunk is a single linear per-partition DMA.
    wt = np.ascontiguousarray(
        Wqkv.reshape(NCHUNK, 128, DT, 128).transpose(0, 3, 2, 1)
        .reshape(NCHUNK, 128, DT * 128)
    ).astype(bf16)                                          # [48,128,2048]
    bqt = np.ascontiguousarray(b_qkv.reshape(NCHUNK, 128).T)  # [128,48] f32

    in_maps = []
    for i in range(N_CORES):
        tok = xs_flat[SPC * i:SPC * (i + 1)].reshape(TOK, D)
        xst = np.ascontiguousarray(
            tok.T.reshape(DT, 128, TOK).transpose(1, 0, 2)
            .reshape(128, DT * TOK)).astype(bf16)
        in_maps.append({"xst": xst, "wqkv_t": wt, "bq_t": bqt})
    return in_maps


def make_wout_tiled(Wout):
    Wout = np.asarray(Wout, dtype=np.float32)
    # [eq, p, dt*512+j] = Wout[eq*512+j, dt*128+p]: one linear DMA/quarter
    return np.ascontiguousarray(
        Wout.T.reshape(DT, 128, 4, 512).transpose(2, 1, 0, 3)
        .reshape(4, 128, DT * 512)).astype(ml_dtypes.bfloat16)


def kernel(x, Wqkv, b_qkv, Wout, b_out):
    from concourse import bass_utils

    nc = get_program()
    in_maps = make_in_maps(x, Wqkv, b_qkv)
    wot = make_wout_tiled(Wout)
    for m in in_maps:
        m["wout_t"] = wot

    res = bass_utils.run_bass_kernel_spmd(
        nc, in_maps, core_ids=list(range(N_CORES)))
    outs = [res.results[i]["out"] for i in range(N_CORES)]
    full = np.concatenate(outs, axis=0) + np.asarray(b_out, dtype=np.float32)
    return np.ascontiguousarray(full.reshape(B, NSEG * L, D), dtype=np.float32)
